# revision 1
# baseline (speedup 1.0000x reference)
"""Linear attention (B=2, L=4096, DM=1024, H=16) on 8 trn2 NeuronCores.

Sharding: rows (B*L) split 8 ways -> each core owns 512 rows of each batch
(1024 rows total). Projections, feature map, denominators, V@S and the output
projection are all row-local. The only cross-core term is S = K^T Q per
(batch, head) -- a (64, 64) matrix contracted over all L -> 512KB AllReduce.

Layouts (matmul contracts over the partition dim; computes lhsT.T @ rhs):
 - host pre-transposes activations: XT = X_c^T  [dm, l]
 - q, k natural [l, d]:  lhsT = XT chunk (stationary), rhs = W chunk
 - vT [d, l]:            lhsT = Wv chunk (stationary), rhs = XT chunk
 - S[e, d] += k_tile[:, he].T @ q_tile[:, hd]   (l on partitions)
 - attnT[d, m]: lhsT = S chunk, rhs = vT  -> transposed output feeds the
   output projection as its stationary operand directly (no transposes).
 - biases are added by a K=1 matmul (ones row x bias row) that opens each
   PSUM accumulation group.
Matmul dtype float32r: fp32 bits, truncated-multiply PE mode, 1 cycle/row at
N=512 (4x faster than fp32), ~1.5e-4 rel err measured on hw.
"""
import sys

sys.path.insert(0, "/opt/trn_rl_repo")
import numpy as np

B, L, DM, H = 2, 4096, 1024, 16
D = DM // H  # 64
N_CORES = 8
ROWS = B * L // N_CORES  # 1024 rows per core
RPB = ROWS // B  # 512 rows per batch per core
NT = ROWS // 128  # 8 l-tiles per core (4 per batch)
KC = DM // 128  # 8 contraction chunks

_CACHE = {}


def _build():
    import concourse.bass as bass
    import concourse.mybir as mybir
    import concourse.tile as tile
    from concourse import bacc
    from concourse.masks import make_identity

    dt = mybir.dt
    f32, f32r = dt.float32, dt.float32r
    AFT = mybir.ActivationFunctionType

    nc = bacc.Bacc("TRN2", target_bir_lowering=False, debug=False,
                   num_devices=N_CORES)

    qT_d = nc.dram_tensor("qT", [DM, ROWS], f32r, kind="ExternalInput").ap()
    kT_d = nc.dram_tensor("kT", [DM, ROWS], f32r, kind="ExternalInput").ap()
    vT_d = nc.dram_tensor("vT", [DM, ROWS], f32r, kind="ExternalInput").ap()
    W_d = {w: nc.dram_tensor(w, [DM, DM], f32r, kind="ExternalInput").ap()
           for w in ("Wq", "Wk", "Wv", "Wo")}
    b_d = {b: nc.dram_tensor(b, [1, DM], f32r, kind="ExternalInput").ap()
           for b in ("bq", "bk", "bv", "bo")}
    out_d = nc.dram_tensor("out", [ROWS, DM], f32, kind="ExternalOutput").ap()

    with tile.TileContext(nc) as tc:
        with (
            tc.tile_pool(name="xt", bufs=1) as xt_pool,
            tc.tile_pool(name="w", bufs=6) as w_pool,
            tc.tile_pool(name="act", bufs=1) as act_pool,
            tc.tile_pool(name="tmp", bufs=3) as tmp_pool,
            tc.tile_pool(name="small", bufs=1) as small_pool,
            tc.tile_pool(name="rb", bufs=6) as rb_pool,
            tc.tile_pool(name="ps", bufs=8, space="PSUM") as ps_pool,
            tc.tile_pool(name="dram", bufs=1, space="DRAM") as dram_pool,
        ):
            ones_f = small_pool.tile([1, 512], f32, tag="ones_f", name="ones_f")
            nc.vector.memset(ones_f[:], 1.0)
            ones = small_pool.tile([1, 512], f32r, tag="ones", name="ones")
            nc.vector.tensor_copy(ones[:], ones_f[:])
            ident = small_pool.tile([128, 128], f32, tag="ident", name="ident")
            make_identity(nc, ident[:])
            def load_bias(b):
                t = small_pool.tile([1, DM], f32r, tag="bias", name="bias", bufs=2)
                nc.sync.dma_start(t[:], b_d[b])
                return t

            # ================= projections =================
            def load_xt(x_dram):
                tiles = []
                for kc in range(KC):
                    t = xt_pool.tile([128, ROWS], f32r, tag=f"xt{kc}", name=f"xt{kc}")
                    nc.gpsimd.dma_start(t[:], x_dram[kc * 128:(kc + 1) * 128, :])
                    tiles.append(t)
                return tiles

            def proj_natural(xt_tiles, w_dram, bias, out_tag):
                """out[l, d] = elu(X @ W + b)+1 as 8 tiles [128, 1024] f32r."""
                outs = [act_pool.tile([128, DM], f32r, tag=f"{out_tag}{m}", name=f"{out_tag}{m}")
                        for m in range(NT)]
                for n in range(2):
                    psums = [ps_pool.tile([128, 512], f32, tag="pp", name="pp")
                             for _ in range(NT)]
                    for m in range(NT):
                        nc.tensor.matmul(psums[m][:], ones[:1, :128],
                                         bias[:1, n * 512:(n + 1) * 512],
                                         start=True, stop=False)
                    for kc in range(KC):
                        wt = w_pool.tile([128, 512], f32r, tag="w", name="w")
                        nc.sync.dma_start(
                            wt[:], w_dram[kc * 128:(kc + 1) * 128,
                                          n * 512:(n + 1) * 512])
                        for m in range(NT):
                            nc.tensor.matmul(
                                psums[m][:],
                                xt_tiles[kc][:, m * 128:(m + 1) * 128],
                                wt[:], start=False, stop=(kc == KC - 1))
                    for m in range(NT):
                        mn = tmp_pool.tile([128, 512], f32, tag="mn", name="mn")
                        ex = tmp_pool.tile([128, 512], f32, tag="ex", name="ex")
                        nc.vector.tensor_scalar_min(mn[:], psums[m][:], 0.0)
                        nc.scalar.activation(ex[:], mn[:], AFT.Exp)
                        nc.vector.scalar_tensor_tensor(
                            outs[m][:, n * 512:(n + 1) * 512], psums[m][:],
                            0.0, ex[:], op0=mybir.AluOpType.max,
                            op1=mybir.AluOpType.add)
                return outs

            def proj_vT(xt_tiles, w_dram, bias):
                """vT[d, m] = (X @ Wv + b)^T as 8 tiles [128, ROWS] f32r."""
                outs = [act_pool.tile([128, ROWS], f32, tag=f"vt{t}", name=f"vt{t}")
                        for t in range(KC)]
                for t in range(KC):
                    ps2 = [ps_pool.tile([128, 512], f32, tag="pp", name="pp")
                           for _ in range(2)]
                    for n in range(2):
                        nc.tensor.matmul(
                            ps2[n][:], bias[:1, t * 128:(t + 1) * 128],
                            ones[:1, :512], start=True, stop=False)
                    for kc in range(KC):
                        wt = w_pool.tile([128, 128], f32r, tag="wv", name="wv")
                        nc.sync.dma_start(
                            wt[:], w_dram[kc * 128:(kc + 1) * 128,
                                          t * 128:(t + 1) * 128])
                        for n in range(2):
                            nc.tensor.matmul(
                                ps2[n][:], wt[:],
                                xt_tiles[kc][:, n * 512:(n + 1) * 512],
                                start=False, stop=(kc == KC - 1))
                    for n in range(2):
                        nc.scalar.activation(
                            outs[t][:, n * 512:(n + 1) * 512], ps2[n][:],
                            AFT.Copy)
                return outs

            q_t = proj_natural(load_xt(qT_d), W_d["Wq"], load_bias("bq"), "q")
            k_t = proj_natural(load_xt(kT_d), W_d["Wk"], load_bias("bk"), "k")

            # ============ S = K^T Q per (b, h), local partial ============
            # S4[b*2+g] [64, 512]: heads g*8..g*8+7, 64 cols each, base 0
            S_ps4 = [ps_pool.tile([64, 512], f32, tag="pp", name="S_ps")
                     for _ in range(4)]
            for b in range(B):
                for g in range(2):
                    for idx in range(8):
                        h = g * 8 + idx
                        for lc in range(NT // B):
                            m = b * (NT // B) + lc
                            nc.tensor.matmul(
                                S_ps4[b * 2 + g][0:64, idx * 64:(idx + 1) * 64],
                                k_t[m][:, h * 64:(h + 1) * 64],
                                q_t[m][:, h * 64:(h + 1) * 64],
                                start=(lc == 0), stop=(lc == NT // B - 1))

            # ---- AllReduce S partials across the 8 cores ----
            cc_in = dram_pool.tile([64, 2048], f32, tag="ccin", name="ccin")
            cc_out = dram_pool.tile([64, 2048], f32, tag="ccout", name="ccout")
            for j in range(4):
                ssb = small_pool.tile([64, 512], f32, tag=f"ssb{j}", name="ssb")
                nc.vector.tensor_copy(ssb[:], S_ps4[j][:])
                nc.sync.dma_start(cc_in[:, j * 512:(j + 1) * 512], ssb[:])
            nc.gpsimd.collective_compute(
                "AllReduce", mybir.AluOpType.add,
                replica_groups=[list(range(N_CORES))],
                ins=[cc_in[:].opt()], outs=[cc_out[:].opt()])

            # v-projection overlaps the AllReduce (independent of S)
            vT_t = proj_vT(load_xt(vT_d), W_d["Wv"], load_bias("bv"))

            # ---- denominators (local, overlaps the collective) ----
            recipT = small_pool.tile([16, ROWS], f32, tag="recipT", name="recipT")
            for m in range(NT):
                den = tmp_pool.tile([128, 16], f32, tag="den", name="den")
                for half in range(2):
                    prod = tmp_pool.tile([128, 512], f32, tag="mn", name="prod")
                    nc.vector.tensor_mul(prod[:], q_t[m][:, half * 512:(half + 1) * 512],
                                         k_t[m][:, half * 512:(half + 1) * 512])
                    nc.vector.reduce_sum(
                        den[:, half * 8:(half + 1) * 8],
                        prod[:].rearrange("p (h d) -> p h d", h=8),
                        axis=mybir.AxisListType.X)
                dent = ps_pool.tile([16, 128], f32, tag="pp", name="dent")
                nc.tensor.transpose(dent[:], den[:], ident[:])
                nc.vector.tensor_scalar_add(
                    recipT[:, m * 128:(m + 1) * 128], dent[:], 1e-6)
            nc.vector.reciprocal(recipT[:], recipT[:])

            # ==== attnT = ((V @ S) / denom)^T via block-diag head pairs ====
            attnT = [act_pool.tile([128, ROWS], f32r, tag=f"q{t}", name=f"attnT{t}")
                     for t in range(KC)]
            for b in range(B):
                for t in range(KC):
                    h0, h1 = 2 * t, 2 * t + 1
                    bd = rb_pool.tile([128, 128], f32, tag="bd", name="bd", bufs=6)
                    nc.vector.memset(bd[:], 0.0)
                    for j, h in ((0, h0), (1, h1)):
                        nc.sync.dma_start(
                            bd[j * 64:(j + 1) * 64, j * 64:(j + 1) * 64],
                            cc_out[:, (b * 2 + h // 8) * 512
                                   + (h % 8) * 64:(b * 2 + h // 8) * 512
                                   + (h % 8) * 64 + 64])
                    row0 = rb_pool.tile([1, RPB], f32, tag="row", name="row", bufs=6)
                    row1 = rb_pool.tile([1, RPB], f32, tag="row", name="row", bufs=6)
                    nc.sync.dma_start(row0[:], recipT[h0:h0 + 1, b * RPB:(b + 1) * RPB])
                    nc.sync.dma_start(row1[:], recipT[h1:h1 + 1, b * RPB:(b + 1) * RPB])
                    rb0 = rb_pool.tile([128, RPB], f32, tag="rb", name="rb", bufs=6)
                    rb1 = rb_pool.tile([128, RPB], f32, tag="rb", name="rb", bufs=6)
                    nc.gpsimd.partition_broadcast(rb0[:], row0[:])
                    nc.gpsimd.partition_broadcast(rb1[:], row1[:])
                    ps = ps_pool.tile([128, RPB], f32, tag="pp", name="pa")
                    nc.tensor.matmul(ps[:], bd[:],
                                     vT_t[t][:, b * RPB:(b + 1) * RPB],
                                     start=True, stop=True)
                    nc.vector.tensor_mul(
                        attnT[t][0:64, b * RPB:(b + 1) * RPB],
                        ps[0:64, :], rb0[0:64, :])
                    nc.vector.tensor_mul(
                        attnT[t][64:128, b * RPB:(b + 1) * RPB],
                        ps[64:128, :], rb1[64:128, :])

            # ================= output projection =================
            bias_o = load_bias("bo")
            for n in range(2):
                psums = [ps_pool.tile([128, 512], f32, tag="pp", name="pp")
                         for _ in range(NT)]
                for m in range(NT):
                    nc.tensor.matmul(psums[m][:], ones[:1, :128],
                                     bias_o[:1, n * 512:(n + 1) * 512],
                                     start=True, stop=False)
                for kc in range(KC):
                    wt = w_pool.tile([128, 512], f32r, tag="w", name="w")
                    nc.sync.dma_start(
                        wt[:], W_d["Wo"][kc * 128:(kc + 1) * 128,
                                         n * 512:(n + 1) * 512])
                    for m in range(NT):
                        nc.tensor.matmul(
                            psums[m][:],
                            attnT[kc][:, m * 128:(m + 1) * 128],
                            wt[:], start=False, stop=(kc == KC - 1))
                for m in range(NT):
                    ot = tmp_pool.tile([128, 512], f32, tag="mn", name="ot")
                    nc.scalar.activation(ot[:], psums[m][:], AFT.Copy)
                    nc.gpsimd.dma_start(
                        out_d[m * 128:(m + 1) * 128, n * 512:(n + 1) * 512],
                        ot[:])

    nc.compile()
    return nc


def _get_nc():
    if "nc" not in _CACHE:
        _CACHE["nc"] = _build()
    return _CACHE["nc"]


def kernel(query, key, value, Wq, bq, Wk, bk, Wv, bv, Wo, bo, **kw):
    from concourse.bass_utils import run_bass_kernel_spmd

    nc = _get_nc()
    query = np.asarray(query, dtype=np.float32)
    key = np.asarray(key, dtype=np.float32)
    value = np.asarray(value, dtype=np.float32)
    weights = {n: np.ascontiguousarray(np.asarray(w, np.float32))
               for n, w in (("Wq", Wq), ("Wk", Wk), ("Wv", Wv), ("Wo", Wo))}
    biases = {n: np.ascontiguousarray(np.asarray(b, np.float32).reshape(1, DM))
              for n, b in (("bq", bq), ("bk", bk), ("bv", bv), ("bo", bo))}

    in_maps = []
    for c in range(N_CORES):
        sl = slice(c * RPB, (c + 1) * RPB)
        m = {
            "qT": np.ascontiguousarray(
                np.concatenate([query[b, sl] for b in range(B)], 0).T),
            "kT": np.ascontiguousarray(
                np.concatenate([key[b, sl] for b in range(B)], 0).T),
            "vT": np.ascontiguousarray(
                np.concatenate([value[b, sl] for b in range(B)], 0).T),
        }
        m.update(weights)
        m.update(biases)
        in_maps.append(m)

    res = run_bass_kernel_spmd(nc, in_maps, list(range(N_CORES)), **kw)
    out = np.empty((B, L, DM), np.float32)
    for c in range(N_CORES):
        o = res.results[c]["out"]
        for b in range(B):
            out[b, c * RPB:(c + 1) * RPB] = o[b * RPB:(b + 1) * RPB]
    if kw:
        return out, res
    return out

